# revision 104
# baseline (speedup 1.0000x reference)
"""MinHash sketch kernel for Trainium2 (8 NeuronCores, Bass/Tile).

Computes: sketch = segment_min(x @ hash_matrices.T, batch) over 512 segments,
with empty segments set to 0.  x: [N, 256] f32, batch: [N] sorted int64,
hash_matrices: [128, 256] f32 -> out [512, 128] f32.

Strategy (data-parallel over nodes):
  * Host sorts nodes by segment and cuts the order into W=8-wide windows,
    distributed contiguously over the 8 cores (padded with repeats of the
    last node - min-neutral - so every core runs the identical program).
    Windows that straddle a segment boundary (~num_segments of them) are
    recomputed exactly on the host and their device minima ignored.
  * Each core's node shard is laid out TRANSPOSED on host ([256, cols]) so the
    contraction dim (features) sits on SBUF partitions; within each compute
    block the columns are interleaved so block-group i = columns {i + gb*j},
    making every min-fold a packed halving.
  * Device per 2048-col block: x streams in 4096-col DMAs (two blocks per
    DMA - each DMA queue serializes dispatch->DGE->transfer, so fewer,
    larger transfers are needed to outpace the PE) alternating the Pool
    SWDGE and SP HWDGE queues; matmul accumulates into TWO independent
    2-bank PSUM half-tiles; the PSUM is drained inline by its only two
    min/copy-capable readers - DVE folds half A (tensor_tensor, one PSUM
    operand + an Act-made fp32 copy of A's upper quarter) while Act copies
    half B to fp16 - so each half-tile frees without waiting on the other.
    The remaining fp16 folds run on DVE at 2 elem/cycle (2x_1p), software-
    pipelined one block behind and batched two blocks per instruction.
    Accumulator is fp16 (abs error <= ~0.03), flushed at program end over
    all three DMA queues in parallel.
  * Host: scatter-min each (core, group) column back to its segment, zero
    empty segments; exact host fixup for boundary windows.
  * No collective needed: group->segment mapping is host-side, so per-core
    partial sketches are min-combined on the host during unsharding.
  * Cost model (TimelineSim): ~66 us/core; PE-bound (~80% busy) at the
    1 column/cycle matmul rate, DVE ~70%, DMA ~76%.

Precision/speed scheme for the matmul (SCHEME):
  * "e3":    x rounded to float8e3 (e3m4) on host - 1 byte/elem, halving
             HBM traffic vs bf16; H kept in bf16 (its quantization error is
             negligible vs x's).  Single-term mixed-dtype matmul at full PE
             rate.  ~1.1e-2 rel error (tolerance is 2e-2; deterministic).
  * "hilo":  x and H split into bf16 hi+lo pairs on host; 3-term product
             (hi*hi + hi*lo + lo*hi) at full PE rate.  ~4e-6 rel error,
             same DMA bytes as fp32.
  * "f32r":  x, H rounded to FP32R (1-8-11) on host; single-term matmul at
             full PE rate.  ~1.5e-4 rel error.
  * "fp32":  exact fp32 matmul; PE runs at 1/4 rate (2 half-speed passes).
"""

import sys

if "/opt/trn_rl_repo" not in sys.path:
    sys.path.insert(0, "/opt/trn_rl_repo")

import numpy as np

SCHEME = "e3"
N_CORES = 8
W = 8            # nodes per group (reduce_min granularity)
BANK = 512       # PSUM bank width (fp32)
TB = 2048        # columns per full DMA block
NUM_HASHES = 128
FEATURE_DIM = 256

_compiled_cache = {}


def round_fp32r(a):
    """Round-to-nearest-even to FP32R (1-8-11); low 12 mantissa bits zero."""
    b = np.ascontiguousarray(a, dtype=np.float32).view(np.uint32)
    low = b & np.uint32(0xFFF)
    b2 = b & np.uint32(0xFFFFF000)
    up = (low > 0x800) | ((low == 0x800) & (((b2 >> 12) & 1) == 1))
    return (b2 + (up.astype(np.uint32) << 12)).view(np.float32)


def _block_widths(cols):
    """Deterministic block decomposition, shared by host layout and device
    program: small ramp-in blocks (PE starts sooner), full TB-wide blocks,
    then a descending tail so the post-final-DMA compute tail is short."""
    rest = cols
    block_widths = [1024] if rest > 8 * TB else []
    rest -= block_widths[0] if block_widths else 0
    while rest > 3 * 1024 + 128:
        block_widths.append(TB)
        rest -= TB
    for piece in (1024, 512, 512, 512, 512, 256):
        if rest >= piece:
            block_widths.append(piece)
            rest -= piece
        if rest == 0:
            break
    if rest:  # remainder (multiple of W): keep small ones as their own tiny
        # last block - the post-final-matmul tail chain is then minimal
        if rest > 128 and block_widths and block_widths[-1] + rest <= TB:
            block_widths[-1] += rest
        else:
            block_widths.append(rest)
    assert sum(block_widths) == cols, (cols, block_widths)
    return block_widths


def _build_program(cols, scheme):
    """Build + compile the single-core Bass program for a shard of `cols`
    node-columns (cols % TB == 0)."""
    import concourse.bacc as bacc
    import concourse.mybir as mybir
    import concourse.tile as tile

    nc = bacc.Bacc("TRN2", target_bir_lowering=False, debug=False,
                   num_devices=N_CORES)

    assert cols % W == 0
    n_groups = cols // W
    block_widths = _block_widths(cols)

    if scheme == "hilo":
        xdt = hdt = mybir.dt.bfloat16
        x_names = ["xhi", "xlo"]
        h_names = ["hhi", "hlo"]
    elif scheme == "e3":
        xdt = mybir.dt.float8e3
        hdt = mybir.dt.bfloat16
        x_names = ["x8"]
        h_names = ["hb"]
    else:
        xdt = hdt = mybir.dt.float32r if scheme == "f32r" else mybir.dt.float32
        x_names = ["xt"]
        h_names = ["ht"]

    x_in = {n: nc.dram_tensor(n, [FEATURE_DIM, cols], xdt,
                              kind="ExternalInput").ap() for n in x_names}
    h_in = {n: nc.dram_tensor(n, [FEATURE_DIM, NUM_HASHES], hdt,
                              kind="ExternalInput").ap() for n in h_names}
    acc_out = nc.dram_tensor("acc", [NUM_HASHES, n_groups], mybir.dt.float16,
                             kind="ExternalOutput").ap()

    with tile.TileContext(nc) as tc:
        with (
            tc.tile_pool(name="singles", bufs=1) as singles,
            tc.tile_pool(name="xtiles", bufs=3) as xtiles,
            tc.tile_pool(name="cp", bufs=3) as cp_pool,
            tc.tile_pool(name="scr", bufs=3) as scr_pool,
            tc.tile_pool(name="psum", bufs=4, space="PSUM") as psum,
        ):
            acc_sb = singles.tile([128, n_groups], mybir.dt.float16)
            h_sb = {}
            for n in h_names:
                t = singles.tile([128, 2, NUM_HASHES], hdt, tag=f"h_{n}")
                # first thing on the SP queue: x block 0 dispatches on the
                # Pool SWDGE queue in parallel
                nc.sync.dma_start(
                    out=t[:, :, :],
                    in_=h_in[n].rearrange("(c p) m -> p c m", c=2))
                h_sb[n] = t

            # tiny warm-up matmul at t~0: starts the PE p-state ramp clock
            # early so the real matmul stream runs at full rate
            warm = singles.tile([1, 8], mybir.dt.float32, tag="warm")
            nc.scalar.memzero(warm)

            # small tail blocks get their x prefetched mid-stream in one DMA:
            # per-block dispatch at the end (~2us serial each) would stall
            # the end of the PE stream
            tail_cols = 0
            for tb in reversed(block_widths):
                if tb > 512:
                    break
                tail_cols += tb
            tail_col0 = cols - tail_cols
            x_tail = {}
            if tail_cols:
                for n in x_names:
                    xt_pre = singles.tile([128, 2, tail_cols], xdt,
                                          tag=f"xt_{n}")
                    x_tail[n] = xt_pre


            # (weight tensor, chunk, rhs tensor) per accumulation term
            if scheme == "hilo":
                phases = [("hhi", 0, "xhi"), ("hhi", 1, "xhi"),
                          ("hlo", 0, "xhi"), ("hlo", 1, "xhi"),
                          ("hhi", 0, "xlo"), ("hhi", 1, "xlo")]
            else:  # single-term schemes ("e3", "f32r", "fp32")
                phases = [(h_names[0], 0, x_names[0]),
                          (h_names[0], 1, x_names[0])]

            def dve_folds(cpB, g0, gb, tb, scr):
                # pipelined one block behind: fold B's fp16 copy (2x_1p),
                # cross-fold with the already-folded A half, then the final
                # halving into acc.  q = tb/4 elements per f1 output.
                q = tb // 4
                nc.vector.tensor_tensor(
                    out=scr[:, q:2 * q], in0=cpB[:, 0:q],
                    in1=cpB[:, q:2 * q], op=mybir.AluOpType.min)
                nc.vector.tensor_tensor(
                    out=scr[:, 2 * q:3 * q], in0=scr[:, 0:q],
                    in1=scr[:, q:2 * q], op=mybir.AluOpType.min)
                nc.vector.tensor_tensor(
                    out=acc_sb[:, g0:g0 + gb], in0=scr[:, 2 * q:2 * q + gb],
                    in1=scr[:, 2 * q + gb:3 * q], op=mybir.AluOpType.min)

            def pair_folds(cpD, scrD, g0):
                # same fold network for TWO 2048-blocks per instruction via
                # [128, 2, n] batched APs: the b dim rides along, halving
                # DVE's per-instruction overhead.  2x_1p still applies (all
                # fp16, last dim packed).
                cpv = cpD.rearrange("p (b h) -> p b h", b=2)
                av = scrD[:, 0:1024].rearrange("p (b h) -> p b h", b=2)
                f1b = scrD[:, 1024:2048].rearrange("p (b h) -> p b h", b=2)
                f2o = scrD[:, 2048:3072].rearrange("p (b h) -> p b h", b=2)
                nc.vector.tensor_tensor(
                    out=f1b, in0=cpv[:, :, 0:512], in1=cpv[:, :, 512:1024],
                    op=mybir.AluOpType.min)
                nc.vector.tensor_tensor(
                    out=f2o, in0=av, in1=f1b, op=mybir.AluOpType.min)
                f2q = scrD[:, 2048:3072].rearrange("p (b t g) -> p b t g",
                                                   b=2, t=2)
                accv = acc_sb[:, g0:g0 + 512].rearrange("p (b g) -> p b g",
                                                        b=2)
                nc.vector.tensor_tensor(
                    out=accv, in0=f2q[:, :, 0, :], in1=f2q[:, :, 1, :],
                    op=mybir.AluOpType.min)

            # group non-tail blocks into 4096-col DMA units (two compute
            # blocks per DMA): each DMA queue serializes dispatch->DGE->
            # transfer (~2.5us + transfer), so fewer, bigger DMAs are the
            # only way two queues can outrun the PE's 1.2 cols/ns
            groups = []           # (gcol0, gwidth)
            blk_group = {}        # bi -> group index
            c0 = 0
            for bi, tb in enumerate(block_widths):
                if c0 >= cols - (tail_cols or 0) and tail_cols:
                    c0 += tb
                    continue
                if (bi < 2 or not groups
                        or groups[-1][1] + tb > 2 * TB
                        or groups[-1][1] >= 2 * TB):
                    groups.append((c0, tb))
                else:
                    groups[-1] = (groups[-1][0], groups[-1][1] + tb)
                blk_group[bi] = len(groups) - 1
                c0 += tb

            col0 = 0
            pending = None
            pair = None
            g_tiles = {}
            for bi, tb in enumerate(block_widths):
                sl = slice(col0, col0 + tb)
                gb = tb // W
                bank_widths = [min(BANK, tb - k * BANK)
                               for k in range(-(-tb // BANK))]
                if bi == 6 and tail_cols:
                    # mid-stream tail prefetch (dispatch slack exists by now)
                    for n in x_names:
                        nc.sync.dma_start(
                            out=x_tail[n][:, :, :],
                            in_=x_in[n].rearrange("(c p) n -> p c n",
                                                  c=2)[:, :, tail_col0:])
                in_tail = tail_cols and col0 >= tail_col0
                if in_tail:
                    loc = col0 - tail_col0
                    x_sb = {n: x_tail[n][:, :, loc:loc + tb]
                            for n in x_names}
                else:
                    gi = blk_group[bi]
                    gcol0, gw = groups[gi]
                    if gi not in g_tiles:
                        tiles = {}
                        for n in x_names:
                            t = xtiles.tile([128, 2, 2 * TB], xdt,
                                            tag=f"x_{n}")
                            # one DMA for the whole group (both 128-row
                            # chunks), alternating Pool SWDGE / SP HWDGE
                            q = nc.gpsimd if gi % 2 == 0 else nc.sync
                            q.dma_start(
                                out=t[:, :, :gw],
                                in_=x_in[n].rearrange(
                                    "(c p) n -> p c n",
                                    c=2)[:, :, gcol0:gcol0 + gw])
                            tiles[n] = t
                        g_tiles[gi] = tiles
                    loc = col0 - gcol0
                    x_sb = {n: g_tiles[gi][n][:, :, loc:loc + tb]
                            for n in x_names}

                # two independent PSUM half-tiles per block: DVE's fold-1
                # reads only A and Act's copy only B, so each is released
                # as soon as its single reader finishes (a shared tile would
                # wait for both and stall the PE on the slower one)
                hv_a = psum.tile([128, TB // 2], mybir.dt.float32, tag="hv")
                if tb > 512:
                    hv_b = psum.tile([128, TB // 2], mybir.dt.float32,
                                     tag="hv")
                else:
                    hv_b = None
                if col0 == 0:
                    # warm-up: result discarded by the first real start=True
                    nc.tensor.matmul(hv_a[0:1, 0:8], warm[0:1, 0:1],
                                     warm[0:1, 0:8], start=True, stop=True,
                                     skip_group_check=True)
                half = tb // 2
                for p, (hn, chunk, xn) in enumerate(phases):
                    for k, bw in enumerate(bank_widths):
                        c = k * BANK
                        ksl = slice(c, c + bw)
                        if hv_b is None or c < half:
                            dst = hv_a[:, c:c + bw]
                        else:
                            dst = hv_b[:, c - half:c - half + bw]
                        nc.tensor.matmul(dst, h_sb[hn][:, chunk, :],
                                         x_sb[xn][:, chunk, ksl],
                                         start=(p == 0),
                                         stop=(p == len(phases) - 1))

                g0 = col0 // W
                if pending is not None:
                    if pending[0] == "pair":
                        pair_folds(*pending[1:])
                    else:
                        dve_folds(*pending[1:])
                    pending = None
                if tb <= 512:
                    # small tail blocks: one DVE reduce straight from PSUM
                    # (shortest possible post-final-matmul tail chain)
                    nc.vector.tensor_reduce(
                        out=acc_sb[:, g0:g0 + gb],
                        in_=hv_a[:, :tb].rearrange("p (w g) -> p g w", w=W),
                        axis=mybir.AxisListType.X,
                        op=mybir.AluOpType.min,
                    )
                else:
                    # Min-fold tree over the block (host interleaves columns
                    # so block-group i = columns {i + gb*j}; min is
                    # associative so the tree can be reassociated freely).
                    # The PSUM must be drained within ~1.7us or the 2-deep
                    # PSUM pool stalls PE, and only DVE has elementwise min
                    # on TRN2 - so the two PSUM readers run inline in
                    # parallel: DVE folds the left half directly (fp32, 1
                    # elem/cycle), Act copies the right half to fp16.  The
                    # remaining fp16 folds (2 elem/cycle via 2x_1p) are
                    # software-pipelined one block behind, and batched two
                    # blocks per instruction where possible (pair_folds).
                    q = tb // 4
                    if pair is not None:
                        cpD, scrD, g0p = pair
                        cpA = cp_pool.tile([128, 512], mybir.dt.float32,
                                           tag="cpA")
                        nc.scalar.copy(out=cpA[:, :q], in_=hv_a[:, q:half])
                        nc.vector.tensor_tensor(
                            out=scrD[:, 512:1024], in0=hv_a[:, 0:q],
                            in1=cpA[:, :q], op=mybir.AluOpType.min)
                        nc.scalar.copy(out=cpD[:, 1024:2048],
                                       in_=hv_b[:, 0:half])
                        pending = ("pair", cpD, scrD, g0p)
                        pair = None
                    elif (tb == TB and bi + 1 < len(block_widths)
                          and block_widths[bi + 1] == TB):
                        scrD = scr_pool.tile([128, 3072], mybir.dt.float16,
                                             tag="scrD")
                        cpD = cp_pool.tile([128, 2048], mybir.dt.float16,
                                           tag="cpD")
                        cpA = cp_pool.tile([128, 512], mybir.dt.float32,
                                           tag="cpA")
                        nc.scalar.copy(out=cpA[:, :q], in_=hv_a[:, q:half])
                        nc.vector.tensor_tensor(
                            out=scrD[:, 0:512], in0=hv_a[:, 0:q],
                            in1=cpA[:, :q], op=mybir.AluOpType.min)
                        nc.scalar.copy(out=cpD[:, 0:1024],
                                       in_=hv_b[:, 0:half])
                        pair = (cpD, scrD, g0)
                    else:
                        scr = scr_pool.tile([128, (TB * 3) // 4],
                                            mybir.dt.float16, tag="scr")
                        cpB = cp_pool.tile([128, TB // 2], mybir.dt.float16,
                                           tag="cp")
                        cpA = cp_pool.tile([128, 512], mybir.dt.float32,
                                           tag="cpA")
                        nc.scalar.copy(out=cpA[:, :q], in_=hv_a[:, q:half])
                        nc.vector.tensor_tensor(
                            out=scr[:, 0:q], in0=hv_a[:, 0:q],
                            in1=cpA[:, :q], op=mybir.AluOpType.min)
                        nc.scalar.copy(out=cpB[:, :half],
                                       in_=hv_b[:, 0:half])
                        pending = ("solo", cpB, g0, gb, tb, scr)
                col0 += tb
            if pending is not None:
                if pending[0] == "pair":
                    pair_folds(*pending[1:])
                else:
                    dve_folds(*pending[1:])

            # End-only flushes (mid-run flushes head-of-line-block whichever
            # queue they wait on) split across the three DMA queues, with a
            # tiny last range so the final serial chain is short.
            gA = (9 * n_groups // 20) & ~7
            gB = (9 * n_groups // 10) & ~7
            gC = (97 * n_groups // 100) & ~7
            nc.gpsimd.dma_start(out=acc_out[:, :gA], in_=acc_sb[:, :gA])
            nc.scalar.dma_start(out=acc_out[:, gA:gB], in_=acc_sb[:, gA:gB])
            nc.sync.dma_start(out=acc_out[:, gB:gC], in_=acc_sb[:, gB:gC])
            nc.sync.dma_start(out=acc_out[:, gC:], in_=acc_sb[:, gC:])

    nc.compile()
    return nc


def kernel(x, batch, num_segments, hash_matrices):
    import ml_dtypes
    from concourse import bass_utils

    x = np.ascontiguousarray(np.asarray(x), dtype=np.float32)
    batch = np.asarray(batch).astype(np.int64).ravel()
    num_segments = int(num_segments)
    hm = np.asarray(hash_matrices, dtype=np.float32)

    assert x.shape[1] == FEATURE_DIM and hm.shape == (NUM_HASHES, FEATURE_DIM)

    # --- host: window construction -----------------------------------------
    # Sort nodes by segment, pad to a uniform per-core column count with
    # repeats of the last node (same segment -> min-neutral), and cut the
    # order into fixed W-wide windows.  A window whose nodes all share one
    # segment is reduced on device; the ~num_segments windows that straddle
    # a segment boundary are recomputed exactly on the host (tiny).
    n_nodes = batch.shape[0]
    counts = np.bincount(batch, minlength=num_segments)
    order = np.argsort(batch, kind="stable")  # contiguous runs per segment

    gpc = -(-(-(-n_nodes // N_CORES)) // W)   # ceil(ceil(n/8)/W)
    cols = gpc * W
    n_pad = cols * N_CORES - n_nodes
    ord_pad = np.concatenate([order, np.full(n_pad, order[-1], dtype=np.int64)])
    idx = ord_pad.reshape(N_CORES, cols)

    bs = batch[ord_pad].reshape(N_CORES, gpc, W)   # sorted segment per slot
    pure = bs[:, :, 0] == bs[:, :, -1]
    grp_seg = np.where(pure, bs[:, :, 0], -1)      # [N_CORES, gpc]

    # --- host: build per-core shards ---------------------------------------
    # device block-fold layout: within each block, group i (32 consecutive
    # sorted slots) sits at columns {i + gb*j} so halving folds are packed
    perm = np.empty(cols, dtype=np.int64)
    off = 0
    for tb in _block_widths(cols):
        gb = tb // W
        perm[off:off + tb] = (np.arange(off, off + tb, dtype=np.int64)
                              .reshape(gb, W).T.ravel())
        off += tb

    bf16 = ml_dtypes.bfloat16
    in_maps = []
    if SCHEME == "e3":
        e3m4 = ml_dtypes.float8_e3m4
        hb = np.ascontiguousarray(hm.T.astype(bf16))
        for c in range(N_CORES):
            x8 = np.ascontiguousarray(x[idx[c][perm]].T.astype(e3m4))
            in_maps.append({"x8": x8, "hb": hb})
    elif SCHEME == "hilo":
        hhi = hm.T.astype(bf16)
        hlo = (hm.T - hhi.astype(np.float32)).astype(bf16)
        hhi = np.ascontiguousarray(hhi)
        hlo = np.ascontiguousarray(hlo)
        for c in range(N_CORES):
            xt = x[idx[c][perm]].T                   # [256, cols] f32
            xhi = xt.astype(bf16)
            xlo = (xt - xhi.astype(np.float32)).astype(bf16)
            in_maps.append({"xhi": np.ascontiguousarray(xhi),
                            "xlo": np.ascontiguousarray(xlo),
                            "hhi": hhi, "hlo": hlo})
    elif SCHEME == "f32r":
        ht = round_fp32r(np.ascontiguousarray(hm.T))
        for c in range(N_CORES):
            in_maps.append({"xt": round_fp32r(
                np.ascontiguousarray(x[idx[c][perm]].T)), "ht": ht})
    else:
        ht = np.ascontiguousarray(hm.T)
        for c in range(N_CORES):
            in_maps.append({"xt": np.ascontiguousarray(x[idx[c][perm]].T),
                            "ht": ht})

    # --- device ------------------------------------------------------------
    key = (cols, SCHEME)
    if key not in _compiled_cache:
        _compiled_cache[key] = _build_program(cols, SCHEME)
    nc = _compiled_cache[key]

    res = bass_utils.run_bass_kernel_spmd(
        nc, in_maps, core_ids=list(range(N_CORES)), trace=False
    )

    # --- host: combine -----------------------------------------------------
    sketch = np.full((num_segments, NUM_HASHES), np.inf, dtype=np.float32)
    for c in range(N_CORES):
        acc = np.asarray(res.results[c]["acc"]).astype(np.float32)  # [128,gpc]
        valid = grp_seg[c] >= 0
        np.minimum.at(sketch, grp_seg[c][valid], acc.T[valid])
    # exact host fixup for boundary (impure) windows
    fix_nodes = idx.reshape(N_CORES, gpc, W)[~pure].ravel()
    if fix_nodes.size:
        hv_fix = x[fix_nodes] @ hm.T               # [n_fix, 128] fp32
        np.minimum.at(sketch, batch[fix_nodes], hv_fix)
    sketch[counts == 0] = 0.0
    return sketch

